# revision 7
# baseline (speedup 1.0000x reference)
"""Trainium2 Bass kernel for nn_CGATLayer (GNN message passing).

Algorithm (matches reference):
    z = feature @ fc_weight                      # [N, D]
    s = z @ attn[:D];  d = z @ attn[D:]          # per-node scalars
    e[n,j]   = leaky_relu(s[src[n,j]] + d[n])
    alpha[n,j] = sum_k relu(e[n,j] - e[n,k])
    h[n]     = sum_j alpha[n,j] * z[src[n,j]]

Device strategy (8 NeuronCores, SPMD single program):
  - dest nodes sharded: core i owns rows [i*PN, (i+1)*PN)
  - phase A (replicated on every core): build a DRAM "table" [N, 68] bf16:
      cols 0:64  = z row (bf16)
      f32 col 32 = s2  = 0.5 * s          (fp32, bitcast into 2 bf16 slots)
      f32 col 33 = dn2 = -0.5 * d
    The 0.5 scaling exploits positive homogeneity of leaky_relu/relu:
    with e' = e/2,  alpha = sum_k relu(e_j - e_k)
                          = sum_k |e'_j - e'_k| + DEG*e'_j - sum_k e'_k
    which needs only one abs-reduce instead of a relu+reduce pair.
  - phase B: per 128-dest tile, one indirect-DMA gather pulls the 33 table
    rows each dest needs (32 sources + its own row for d), then DVE computes
    e', alpha and the weighted sum.  Indices are host-prepared int32.
"""

from contextlib import ExitStack

import numpy as np

import concourse.bass as bass
import concourse.bacc as bacc
import concourse.tile as tile
from concourse import mybir
from concourse.bass import IndirectOffsetOnAxis

F32 = mybir.dt.float32
BF16 = mybir.dt.bfloat16
I32 = mybir.dt.int32
U16 = mybir.dt.uint16
ALU = mybir.AluOpType
AXL = mybir.AxisListType

N, DEG, IN_DIM, OUT_DIM = 50000, 32, 128, 64
NCORES = 8
NEG_SLOPE = 0.01
P = 128


def build_program(n=N, pn=None, deg=DEG, in_dim=IN_DIM, out_dim=OUT_DIM, cg=7,
                  ncores=NCORES):
    """Build the SPMD Bass program. pn = dest nodes owned by this core."""
    if pn is None:
        pn = n // NCORES
    ntiles = (pn + P - 1) // P
    pnpad = ntiles * P
    row = out_dim + 4            # bf16 units per table row
    rowf = row // 2              # f32 units per table row
    scol = out_dim // 2          # f32 col of s2
    dcol = scol + 1              # f32 col of dn2
    nchunks = (n + P - 1) // P
    ngroups = (nchunks + cg - 1) // cg

    nc = bacc.Bacc("TRN2", num_devices=ncores)
    featT = nc.declare_dram_parameter("featT", [in_dim, n], F32, isOutput=False)
    fc = nc.declare_dram_parameter("fc", [in_dim, out_dim], F32, isOutput=False)
    fcT = nc.declare_dram_parameter("fcT", [out_dim, in_dim], F32, isOutput=False)
    attn2 = nc.declare_dram_parameter("attn2", [out_dim, 2], F32, isOutput=False)
    idx = nc.declare_dram_parameter("idx", [pnpad, deg + 1], I32, isOutput=False)
    h = nc.declare_dram_parameter("h", [pn, out_dim], F32, isOutput=True)
    table = nc.dram_tensor("table", [n, row], U16)

    with tile.TileContext(nc) as tc, ExitStack() as ctx:
        const_pool = ctx.enter_context(tc.tile_pool(name="const", bufs=1))
        ft_pool = ctx.enter_context(tc.tile_pool(name="ft", bufs=3))
        row_pool = ctx.enter_context(tc.tile_pool(name="rowp", bufs=3))
        psA_pool = ctx.enter_context(tc.tile_pool(name="psA", bufs=2, space="PSUM"))
        g_pool = ctx.enter_context(tc.tile_pool(name="g", bufs=4))
        it_pool = ctx.enter_context(tc.tile_pool(name="it", bufs=4))
        sm_pool = ctx.enter_context(tc.tile_pool(name="sm", bufs=4))
        D_pool = ctx.enter_context(tc.tile_pool(name="Dp", bufs=2))
        pr_pool = ctx.enter_context(tc.tile_pool(name="pr", bufs=2))
        h_pool = ctx.enter_context(tc.tile_pool(name="hp", bufs=3))

        # ---- weight prep: R = [fc | 0.5*fc@a1 | -0.5*fc@a2]  [in_dim, out_dim+2]
        fc_sb = const_pool.tile([in_dim, out_dim], F32)
        nc.sync.dma_start(fc_sb[:], fc[:])
        fcT_sb = const_pool.tile([out_dim, in_dim], F32)
        nc.sync.dma_start(fcT_sb[:], fcT[:])
        attn2_sb = const_pool.tile([out_dim, 2], F32)
        nc.sync.dma_start(attn2_sb[:], attn2[:])
        R_sb = const_pool.tile([in_dim, out_dim + 2], F32)
        wsd_ps = psA_pool.tile([in_dim, 2], F32)
        nc.tensor.matmul(out=wsd_ps[:], lhsT=fcT_sb[:], rhs=attn2_sb[:],
                         start=True, stop=True)
        nc.vector.tensor_copy(out=R_sb[:, 0:out_dim], in_=fc_sb[:])
        nc.vector.tensor_scalar(out=R_sb[:, out_dim:out_dim + 1],
                                in0=wsd_ps[:, 0:1], scalar1=0.5, scalar2=None,
                                op0=ALU.mult)
        nc.vector.tensor_scalar(out=R_sb[:, out_dim + 1:out_dim + 2],
                                in0=wsd_ps[:, 1:2], scalar1=-0.5, scalar2=None,
                                op0=ALU.mult)

        # ---- phase A: build table (replicated: every core computes all rows)
        ocols = out_dim + 2
        for gi in range(ngroups):
            c0 = gi * cg
            cn = min(cg, nchunks - c0)
            n0 = c0 * P
            nn = min(n - n0, cn * P)
            ft = ft_pool.tile([P, cg * P], F32, tag="ft")
            nc.sync.dma_start(ft[:, :nn], featT[:, n0:n0 + nn])
            ps = psA_pool.tile([P, cg * ocols], F32, tag="psA")
            for q in range(cn):
                cw = min(P, n - (c0 + q) * P)
                nc.tensor.matmul(out=ps[:cw, q * ocols:(q + 1) * ocols],
                                 lhsT=ft[:, q * P:q * P + cw], rhs=R_sb[:],
                                 start=True, stop=True)
            rowt = row_pool.tile([P, cg * row], U16, tag="rowt")
            ps3 = ps[:].rearrange("p (q f) -> p q f", f=ocols)
            row3 = rowt[:].bitcast(BF16).rearrange("p (q f) -> p q f", f=row)
            rowf3 = rowt[:].bitcast(F32).rearrange("p (q f) -> p q f", f=rowf)
            nc.vector.tensor_copy(out=row3[:, 0:cn, 0:out_dim],
                                  in_=ps3[:, 0:cn, 0:out_dim])
            nc.vector.tensor_copy(out=rowf3[:, 0:cn, scol:dcol + 1],
                                  in_=ps3[:, 0:cn, out_dim:out_dim + 2])
            if nn == cn * P:
                tv = table[n0:n0 + nn, :].rearrange("(q p) f -> p q f", p=P)
                nc.sync.dma_start(out=tv, in_=rowt[:].rearrange("p (q f) -> p q f", f=row)[:, 0:cn, :])
            else:
                for q in range(cn):
                    cw = min(P, n - (c0 + q) * P)
                    nc.sync.dma_start(
                        out=table[(c0 + q) * P:(c0 + q) * P + cw, :],
                        in_=rowt[:cw, q * row:(q + 1) * row])

        # ---- phase B: per-dest-tile gather + attention + weighted sum
        for t in range(ntiles):
            r0 = t * P
            vp = min(P, pn - r0)
            it = it_pool.tile([P, deg + 1], I32, tag="it")
            nc.sync.dma_start(it[:], idx[r0:r0 + P, :])
            g = g_pool.tile([P, (deg + 1) * row], U16, tag="g")
            # One single-offset-per-partition gather per edge slot: the
            # vector-offset (multi-index) DGE path is broken on this stack
            # (XLA disables it too); [P,1]-offset gathers are the reliable
            # primitive.
            for j in range(deg + 1):
                nc.gpsimd.indirect_dma_start(
                    out=g[:, j * row:(j + 1) * row], out_offset=None,
                    in_=table[:],
                    in_offset=IndirectOffsetOnAxis(ap=it[:, j:j + 1], axis=0))
            gf = g[:].bitcast(F32)
            gf3 = gf.rearrange("p (j f) -> p j f", f=rowf)
            s_view = gf3[:, 0:deg, scol:scol + 1]          # [P, deg, 1] = s2[src]
            dn = gf[:, deg * rowf + dcol: deg * rowf + dcol + 1]  # [P,1] own -d/2
            x = sm_pool.tile([P, deg], F32, tag="x")
            nc.vector.tensor_scalar(out=x[:], in0=s_view, scalar1=dn,
                                    scalar2=None, op0=ALU.subtract)
            y = sm_pool.tile([P, deg], F32, tag="y")
            nc.vector.tensor_scalar(out=y[:], in0=x[:], scalar1=NEG_SLOPE,
                                    scalar2=None, op0=ALU.mult)
            e = sm_pool.tile([P, deg], F32, tag="e")
            nc.vector.tensor_tensor(out=e[:], in0=x[:], in1=y[:], op=ALU.max)
            D = D_pool.tile([P, deg * deg], F32, tag="D")
            D3 = D[:].rearrange("p (j k) -> p j k", k=deg)
            nc.vector.tensor_tensor(
                out=D3, in0=e[:].unsqueeze(2).broadcast_to([P, deg, deg]),
                in1=e[:].unsqueeze(1).broadcast_to([P, deg, deg]),
                op=ALU.subtract)
            A = sm_pool.tile([P, deg], F32, tag="A")
            nc.vector.tensor_reduce(out=A[:], in_=D3, axis=AXL.X, op=ALU.add,
                                    apply_absolute_value=True)
            Tn = sm_pool.tile([P, 1], F32, tag="Tn")
            nc.vector.tensor_reduce(out=Tn[:], in_=e[:], axis=AXL.X, op=ALU.add,
                                    negate=True)
            al0 = sm_pool.tile([P, deg], F32, tag="al0")
            nc.vector.tensor_scalar(out=al0[:], in0=e[:], scalar1=float(deg),
                                    scalar2=Tn[:], op0=ALU.mult, op1=ALU.add)
            alpha = sm_pool.tile([P, deg], F32, tag="alpha")
            nc.vector.tensor_tensor(out=alpha[:], in0=al0[:], in1=A[:],
                                    op=ALU.add)
            prod = pr_pool.tile([P, deg * out_dim], F32, tag="prod")
            zv = g[:].bitcast(BF16).rearrange("p (j f) -> p j f", f=row)[:, 0:deg, 0:out_dim]
            ab = alpha[:].unsqueeze(2).broadcast_to([P, deg, out_dim])
            nc.vector.tensor_tensor(
                out=prod[:].rearrange("p (j d) -> p j d", d=out_dim),
                in0=zv, in1=ab, op=ALU.mult)
            hsb = h_pool.tile([P, out_dim], F32, tag="hsb")
            pv = prod[:].rearrange("p (j d) -> p j d", d=out_dim).transpose([0, 2, 1])
            nc.vector.tensor_reduce(out=hsb[:], in_=pv, axis=AXL.X, op=ALU.add)
            nc.sync.dma_start(out=h[r0:r0 + vp, :], in_=hsb[:vp, :])

    nc.compile()
    return nc


def prep_inputs(feature, src_idx, fc_weight, attn_weight, ncores=NCORES):
    """Host-side sharding/layout prep -> per-core input maps."""
    feature = np.asarray(feature, dtype=np.float32)
    src = np.asarray(src_idx).astype(np.int32)
    fcw = np.asarray(fc_weight, dtype=np.float32)
    aw = np.asarray(attn_weight, dtype=np.float32)
    n, in_dim = feature.shape
    out_dim = fcw.shape[1]
    deg = src.shape[1]
    pn = n // ncores
    ntiles = (pn + P - 1) // P
    pnpad = ntiles * P

    featT = np.ascontiguousarray(feature.T)
    fcT = np.ascontiguousarray(fcw.T)
    attn2 = np.ascontiguousarray(
        np.stack([aw[:out_dim, 0], aw[out_dim:, 0]], axis=1))

    in_maps = []
    for c in range(ncores):
        idxp = np.zeros((pnpad, deg + 1), dtype=np.int32)
        idxp[:pn, :deg] = src[c * pn:(c + 1) * pn]
        idxp[:pn, deg] = np.arange(c * pn, (c + 1) * pn, dtype=np.int32)
        in_maps.append({"featT": featT, "fc": fcw, "fcT": fcT,
                        "attn2": attn2, "idx": idxp})
    return in_maps, pn


_prog_cache = {}


def kernel(feature, src_idx, fc_weight, attn_weight):
    from concourse.bass_utils import run_bass_kernel_spmd

    in_maps, pn = prep_inputs(feature, src_idx, fc_weight, attn_weight)
    key = ("v1", feature.shape, pn)
    if key not in _prog_cache:
        _prog_cache[key] = build_program(n=feature.shape[0], pn=pn)
    nc = _prog_cache[key]
    res = run_bass_kernel_spmd(nc, in_maps, list(range(NCORES)))
    h = np.concatenate(
        [np.asarray(res.results[i]["h"]) for i in range(NCORES)], axis=0)
    return np.ascontiguousarray(h, dtype=np.float32)
